# revision 25
# baseline (speedup 1.0000x reference)
"""Weighted two-sided chamfer loss (AutoDecLoss) for Trainium2 -- 8 cores.

Strategy (v2)
-------------
Data-parallel over the batch: core b handles batch element b; the host
combines the per-core partial results.

Single scaled-distance pass: with rw = 1/max(w, 1e-3), the matrix
D'[n,m] = d[n,m] * rw[n] serves BOTH chamfer directions:

  forward : min_m d[n,m] = max(w,1e-3)[n] * min_m D'[n,m]
  backward: min_n d[n,m]/max(w,1e-3)[n] = min_n D'[n,m]

so only ONE [N, M] matmul pass is needed (the baseline ran two).

D' comes off the PE via augmented features (K = 27 bf16 compensated
hi/lo rows, ~1e-4 relative error despite the x^2+y^2-2xy cancellation),
with the X-side features pre-scaled by rw.  Feature construction is done
on the host (numpy) -- it is O(N+M) against the device's O(N*M).

Per n-tile (128 rows) the fp32 PSUM half-tiles [128, 2048] are:
  * converted to bf16 SBUF by the Scalar engine with a fused ReLU
    (which implements the max(d,0) clamp exactly),
  * forward-min-reduced by one DVE tensor_tensor_reduce per n-tile
    (elementwise min of the two m-halves + a per-row min accumulator),
  * backward-accumulated into a [128, 4096] running elementwise min
    (pairwise L1 ops + a serial chain, all bf16 at the DVE's 2x mode;
    the early chain links hide under the ACT-paced conversion stream).
The backward accumulator (a 128-way fold of the full 2048-row min) goes
back to the host, which finishes the partition min, applies weights /
normalization, and averages the batch.  (GPSIMD cannot help: the
hardware lowering rejects min/max tensor_tensor ops on that engine.)
"""

import re

import ml_dtypes
import numpy as np

import concourse.bacc as bacc
import concourse.mybir as mybir
import concourse.tile as tile
from concourse import dve_ops
from concourse.bass_utils import run_bass_kernel_spmd
from concourse.dve_spec import C0, Spec, Src0, Src1, minn
from concourse.dve_table_gen import dve_ver_for

_OP_NAME = "MIN_MIN_REDUCE_ANT"


def _mmr_ref(in0, in1, s0, s1, imm2):
    out = np.minimum(in0.astype(np.float32), in1.astype(np.float32))
    P = out.shape[0]
    body = out.reshape(P, -1)
    seed = np.asarray(s0, np.float32).reshape(-1, 1)
    acc = np.minimum(np.minimum.reduce(body, axis=-1, keepdims=True), seed)
    return out, acc


def get_min_min_reduce():
    """out = min(in0, in1); accum_out = min(seed, row-min(out)).

    Registered custom DVE op (the native TENSOR_TENSOR_REDUCE fails to
    execute on this runtime)."""
    for op in dve_ops.OPS:
        if op.name == _OP_NAME:
            return op
    spec = Spec(body=minn(Src0, Src1), accum=minn, accum_init=C0,
                reference=_mmr_ref)
    ver = dve_ver_for("TRN2")
    probe = dve_ops.DveOp(_OP_NAME, spec, subdim=False, uops_sha={})
    row = dve_ops._CUSTOM_DVE_ROW_BASE + len(dve_ops.OPS)
    dve_ops._SUB_OPCODE_FOR_NAME[_OP_NAME] = row
    shas = {}
    for v in ("v3", "v4"):
        try:
            probe.compile(v)
            shas[v] = probe.uops_sha.get(v)
        except ValueError as e:
            m = re.search(rf"{v}: ([0-9a-f]+)", str(e))
            if not m:
                raise
            shas[v] = m.group(1)
    op = dve_ops.DveOp(_OP_NAME, spec, subdim=False, uops_sha=shas)
    dve_ops.OPS.append(op)
    dve_ops.CUSTOM_DVE_SPECS[_OP_NAME] = spec
    assert ver in shas
    return op

B, N, M = 8, 2048, 4096
NT = N // 128          # 16 n-tiles
MT = M // 128          # 32 m-chunks
HM = M // 2            # 2048
CHAMFER_EPS = 1e-6
MIN_BW = 1e-3
BIG = 3.0e38

F32 = mybir.dt.float32
BF16 = mybir.dt.bfloat16
MIN = mybir.AluOpType.min
AX = mybir.AxisListType.X
RELU = mybir.ActivationFunctionType.Relu

# routing knobs (tuned against TimelineSim)
GPS_BWD_L1 = ()       # (GPSIMD cannot run min ops on trn2 hardware)
GPS_BWD_UP = ()       # (GPSIMD cannot run min ops on trn2 hardware)
GPS_FWD_TILES = ()    # n-tile indices whose forward reduce runs on GPSIMD
DVE_CONV = ()         # (c, g) half-tiles converted by DVE instead of ACT
MIXED_PAIRS = ()      # pair indices whose odd tile's g0 half skips conversion
CONV_BUFS = 6         # converted-tile ring size
N_WARMUP = 6          # PE warmup matmuls to beat the p-state ramp


def build_nc(gps_l1=None, gps_up=None, gps_fwd=None, dve_conv=None,
             conv_bufs=None, n_warmup=None, skip_fwd=False, skip_bwd=False,
             skip_final=False, mixed=None):
    gps_l1 = GPS_BWD_L1 if gps_l1 is None else gps_l1
    gps_up = GPS_BWD_UP if gps_up is None else gps_up
    gps_fwd = GPS_FWD_TILES if gps_fwd is None else gps_fwd
    dve_conv = DVE_CONV if dve_conv is None else dve_conv
    conv_bufs = CONV_BUFS if conv_bufs is None else conv_bufs
    n_warmup = N_WARMUP if n_warmup is None else n_warmup
    mixed = MIXED_PAIRS if mixed is None else mixed
    mmr = get_min_min_reduce()
    nc = bacc.Bacc("TRN2", target_bir_lowering=False, debug=False, num_devices=8)
    xs = nc.dram_tensor("xs", [27, N], BF16, kind="ExternalInput")
    ys = nc.dram_tensor("ys", [27, M], BF16, kind="ExternalInput")
    out_f = nc.dram_tensor("minf", [128, NT], F32, kind="ExternalOutput")
    out_b = nc.dram_tensor("ufin", [128, M], BF16, kind="ExternalOutput")

    with tile.TileContext(nc) as tc:
        with (
            tc.tile_pool(name="feat", bufs=1) as fpool,
            tc.tile_pool(name="small", bufs=1) as spool,
            tc.tile_pool(name="utree", bufs=1) as upool,
        ):
            XS = fpool.tile([27, N], BF16, tag="XS")
            nc.sync.dma_start(XS[:, 0:256], xs[:, 0:256])
            nc.sync.dma_start(XS[:, 256:N], xs[:, 256:N])
            Y = fpool.tile([27, M], BF16, tag="Y")
            for q in range(4):
                qs = slice(q * 1024, (q + 1) * 1024)
                nc.sync.dma_start(Y[:, qs], ys[:, qs])

            # PE warmup operands (zeros)
            wl = spool.tile([2, 128], BF16, tag="wl")
            nc.vector.memset(wl[:], 0.0)
            wr = spool.tile([2, 512], BF16, tag="wr")
            nc.vector.memset(wr[:], 0.0)

            minf = spool.tile([128, NT], F32, tag="minf")
            junk0 = spool.tile([128, HM], BF16, tag="junk0")
            junk1 = spool.tile([128, HM], BF16, tag="junk1")

            us = [upool.tile([128, M], BF16, tag=f"u{g}", name=f"u{g}")
                  for g in range(8)]

            with (
                tc.tile_pool(name="psum_main", bufs=2, space="PSUM") as mpool,
                tc.tile_pool(name="conv", bufs=conv_bufs) as cpool,
            ):
                # warmup: run the PE on zeros while the DMAs stream in
                psw = mpool.tile([128, HM], F32, tag="ps")
                for _ in range(n_warmup):
                    nc.tensor.matmul(psw[:, 0:512], wl[:], wr[:], start=True,
                                     stop=True)

                cprev = None
                for c in range(NT):
                    # odd tiles of "mixed" pairs: the g0 half stays raw in
                    # PSUM (read by fwd TTR and a 1x mixed backward min),
                    # skipping its ACT conversion.
                    is_mixed = (c % 2 == 1) and (c // 2) in mixed
                    C = cpool.tile([128, M], BF16, tag="C")
                    lhsT = XS[:, c * 128:(c + 1) * 128]
                    ps_raw = None
                    for g in range(2):
                        ps = mpool.tile([128, HM], F32, tag="ps")
                        for k in range(HM // 512):
                            f0 = g * HM + k * 512
                            nc.tensor.matmul(ps[:, k * 512:(k + 1) * 512],
                                             lhsT, Y[:, f0:f0 + 512],
                                             start=True, stop=True)
                        # fp32 PSUM -> bf16 SBUF with fused max(.,0)
                        cs = slice(g * HM, (g + 1) * HM)
                        if is_mixed and g == 0:
                            ps_raw = ps
                        elif (c, g) in dve_conv:
                            nc.vector.tensor_scalar_max(C[:, cs], ps[:], 0.0)
                        else:
                            nc.scalar.activation(C[:, cs], ps[:], RELU)
                    # forward: per-row min over all 4096 m
                    if skip_fwd:
                        pass
                    elif is_mixed:
                        nc.vector._custom_dve(
                            mmr, out=junk0[:] if c % 2 == 0 else junk1[:],
                            in0=ps_raw[:], in1=C[:, HM:M], s0=BIG,
                            accum_out=minf[:, c:c + 1])
                    elif c in gps_fwd:
                        nc.gpsimd.tensor_reduce(minf[:, c:c + 1], C[:],
                                                axis=AX, op=MIN)
                    else:
                        nc.vector._custom_dve(
                            mmr, out=junk0[:] if c % 2 == 0 else junk1[:],
                            in0=C[:, 0:HM], in1=C[:, HM:M], s0=BIG,
                            accum_out=minf[:, c:c + 1])
                    # backward L1: elementwise min of n-tile pairs
                    if c % 2 == 1 and not skip_bwd:
                        g = c // 2
                        if is_mixed:
                            # raw-psum half: mixed-dtype 1x min; note the
                            # raw half is NOT clamped at 0 -- clamped later
                            # on the [128, MT] minb result instead.
                            nc.vector.tensor_tensor(
                                us[g][:, 0:HM], ps_raw[:], cprev[:, 0:HM],
                                op=MIN)
                            nc.vector.tensor_tensor(
                                us[g][:, HM:M], C[:, HM:M], cprev[:, HM:M],
                                op=MIN)
                        elif c == NT - 1:
                            # last pair: split into halves so the final
                            # merge + output DMA pipeline tightly
                            for h in range(2):
                                hs = slice(h * HM, (h + 1) * HM)
                                nc.vector.tensor_tensor(
                                    us[g][:, hs], cprev[:, hs], C[:, hs],
                                    op=MIN)
                        else:
                            eng = nc.gpsimd if g in gps_l1 else nc.vector
                            eng.tensor_tensor(us[g][:], cprev[:], C[:],
                                              op=MIN)
                    cprev = C

            # backward upper stage: serial chain over the 8 u tiles.
            # Early chain ops hide under the main loop (u_g arrives every
            # ~2 n-tiles); the last merge is split into m-halves so the
            # final transposes can start on half A while half B finishes.
            accs = [upool.tile([128, M], BF16, tag=f"acc{i % 2}",
                               name=f"acc{i}") for i in range(6)]
            ufin = upool.tile([128, M], BF16, tag="accF", name="ufin")
            chain = [(accs[0], us[0], us[1])]
            for i in range(5):
                chain.append((accs[i + 1], accs[i], us[i + 2]))
            for i, (o, a, b) in enumerate(chain):
                if skip_bwd:
                    break
                if i == 5 and i not in gps_up:
                    for h in range(2):
                        hs = slice(h * HM, (h + 1) * HM)
                        nc.vector.tensor_tensor(o[:, hs], a[:, hs], b[:, hs],
                                                op=MIN)
                else:
                    eng = nc.gpsimd if i in gps_up else nc.vector
                    eng.tensor_tensor(o[:], a[:], b[:], op=MIN)
            if not skip_bwd:
                for h in range(2):
                    hs = slice(h * HM, (h + 1) * HM)
                    nc.vector.tensor_tensor(ufin[:, hs], accs[5][:, hs],
                                            us[7][:, hs], op=MIN)
                    nc.sync.dma_start(out_b[:, hs], ufin[:, hs])

            if skip_bwd or skip_final:
                nc.vector.memset(ufin[:], 0.0)
            if skip_fwd:
                nc.vector.memset(minf[:], 0.0)
            nc.sync.dma_start(out_f[:], minf[:])
            if skip_bwd or skip_final:
                nc.sync.dma_start(out_b[:], ufin[:])

    nc.compile()
    return nc


_NC_CACHE = {}


def get_nc():
    if "nc" not in _NC_CACHE:
        _NC_CACHE["nc"] = build_nc()
    return _NC_CACHE["nc"]


def _bf16_pair(a):
    hi = a.astype(ml_dtypes.bfloat16)
    lo = (a - hi.astype(np.float32)).astype(ml_dtypes.bfloat16)
    return hi, lo


def make_in_maps(points, decoded_points, decoded_weights):
    in_maps = []
    metas = []
    for b in range(B):
        x = np.asarray(decoded_points[b], np.float32)      # [N, 3]
        y = np.asarray(points[b], np.float32)              # [M, 3]
        w = np.asarray(decoded_weights[b], np.float32)     # [N]
        wc = np.maximum(w, MIN_BW)

        A = np.empty((9, N), np.float32)
        A[0:3] = (x * x).T
        A[3:6] = (-2.0 * x).T
        A[6:9] = 1.0
        A /= wc[None, :]
        Ah, Al = _bf16_pair(A)
        XS27 = np.concatenate([Ah, Ah, Al], axis=0)        # [27, N]

        Bf = np.empty((9, M), np.float32)
        Bf[0:3] = 1.0
        Bf[3:6] = y.T
        Bf[6:9] = (y * y).T
        Bh, Bl = _bf16_pair(Bf)
        Y27 = np.concatenate([Bh, Bl, Bh], axis=0)         # [27, M]

        in_maps.append({"xs": np.ascontiguousarray(XS27),
                        "ys": np.ascontiguousarray(Y27)})
        metas.append((w, wc))
    return in_maps, metas


def kernel(points, decoded_points, decoded_weights):
    nc = get_nc()
    in_maps, metas = make_in_maps(points, decoded_points, decoded_weights)
    res = run_bass_kernel_spmd(nc, in_maps, core_ids=list(range(B)))
    losses = np.empty(B, np.float64)
    for b in range(B):
        w, wc = metas[b]
        minf = np.asarray(res.results[b]["minf"], np.float64)  # [128, NT]
        ufin = np.asarray(res.results[b]["ufin"], np.float32)  # [128, M]
        # minf[p, c] = min_m D'[n=c*128+p, m]; clamp at 0 here (the raw
        # mixed-pair halves skip the device-side relu).
        mf = np.maximum(minf.T.reshape(-1), 0.0)               # [N] n-major
        wsum = max(float(w.sum()), CHAMFER_EPS)
        fwd = float((w * wc * mf).sum()) / wsum
        bwd = float(np.maximum(ufin.min(axis=0), 0.0).mean())
        losses[b] = fwd + bwd
    return np.asarray(losses.mean(), dtype=np.float32)
